# revision 22
# baseline (speedup 1.0000x reference)
"""DAHead (dual attention head) Trainium2 Bass kernel — v2.

Sharding: 8 cores = (batch b in 0..3) x (image half: rows 0-31 / 32-63).
v2 layout/schedule changes vs v1:
  - conv1 runs in bf16 with row-aligned chunks and writes the un-padded
    [C1, 34, 64] ypx/ycx layout directly via strided epilogues (no separate
    padded y buffers, no extraction DMAs).
  - ONE merged kd pair-collective (fp8 dT + bf16 k payload) instead of two,
    overlapped with conv_c1; gram collective overlaps PAM.
  - PAM/CAM run on the 2176-query space (no width padding); epilogues write
    masked + strided into the 66-wide padded p/c buffers for conv2.
  - z broadcast via a rank-1 ones matmul into PSUM (no DRAM bounce).
  - All parameters packed host-side into a handful of large DMAs.
"""

import os

import numpy as np

import concourse.bass as bass
import concourse.mybir as mybir
import concourse.tile as tile
from concourse.bass_utils import run_bass_kernel_spmd
from concourse.vector_clock import ScopedClock

FP32 = mybir.dt.float32
FP32R = mybir.dt.float32r
BF16 = mybir.dt.bfloat16
F8 = mybir.dt.float8e4
AF = mybir.ActivationFunctionType

NCORES = 8
B, CIN, H, W = 4, 512, 64, 64
C1 = 128          # conv1 out channels
C8 = 16           # q/k channels
CO = 32           # conv2 out channels
WP = W + 2        # padded width 66
XR = 36           # x rows in padded buffer (34 ext rows + 2 conv pad)
PR = 34           # ext rows (32 + 1 halo each side)
NQ = PR * W       # 2176 query positions (un-padded ext space)
NLOC = 32 * W     # 2048 valid key positions per core
NFULL = 64 * W    # 4096 total key positions
EPS = 1e-5

# row-aligned conv chunks over the 34 ext rows: (r0, nrows)
CHR = [(0, 7), (7, 7), (14, 7), (21, 7), (28, 6)]
# conv2 output rows (32 valid rows)
CH2 = [(0, 7), (7, 7), (14, 7), (21, 7), (28, 4)]
# PAM chunks over the flat 2176 query space (all >=256 wide: no fp32r
# small-moving penalty; 448=7 rows, 384=6 rows)
CHP = [(0, 448), (448, 448), (896, 448), (1344, 448), (1792, 384)]

# per-half collective payload: [dT fp8 8*128*128 | k fp32r 16*1024]
DT_BYTES = 8 * C1 * C1            # 131072
K_BYTES = C8 * 1024 * 4           # 65536
MH_BYTES = DT_BYTES + K_BYTES     # 196608

PAIRS = [[0, 1], [2, 3], [4, 5], [6, 7]]

_legalize_counter = [0]


def _patched_drain_and_barrier(self, tick_clock, wait_clock):
    """Tail drain split into single-wait drains (this walrus build encodes at
    most one sync wait per instruction)."""
    drain_inst = self.nc.sync.drain()
    wait_clock.add_sem_waits(
        drain_inst.ins, ScopedClock({None: tick_clock.global_clock})
    )
    si = drain_inst.ins.sync_info
    waits = list(si.on_wait) if si is not None else []
    if len(waits) > 1:
        si.on_wait = waits[:1]
        for i in range(1, len(waits)):
            extra = self.nc.sync.drain()
            extra.ins.sync_info = mybir.SyncInfo(on_wait=[waits[i]], on_update=[])
    self.nc.all_engine_barrier()
    assert self.sems is not None
    popped = self.nc._tile_sem_poison_stack.pop()
    assert popped is self._sem_poison
    self.nc.clear_and_free_semaphores(list(self.sems.allocated().values()))
    self.nc.all_engine_barrier()


tile.TileContext._drain_and_barrier = _patched_drain_and_barrier


def legalize_single_wait(nc):
    """Hoist extra sync waits onto same-engine EventSemaphore instructions so
    every instruction carries at most one wait."""
    n_split = 0
    for fn in nc.m.functions:
        for bb in fn.blocks:
            insts = bb.instructions
            out = []
            changed = False
            for inst in insts:
                si = getattr(inst, "sync_info", None)
                if si is not None and si.on_wait and len(si.on_wait) > 1:
                    waits = list(si.on_wait)
                    for w in waits[:-1]:
                        _legalize_counter[0] += 1
                        out.append(
                            mybir.InstEventSemaphore(
                                name=f"legwait-{_legalize_counter[0]}",
                                engine=inst.engine,
                                ins=[],
                                outs=[],
                                sync_info=mybir.SyncInfo(on_wait=[w], on_update=[]),
                            )
                        )
                        n_split += 1
                    si.on_wait = waits[-1:]
                    changed = True
                out.append(inst)
            if changed:
                insts[:] = out
    return n_split


# fpk (fp32 pack) free-dim layout
FPK_WD = 0            # wdT [128, 128]
FPK_ID = 128          # identity [128, 128]
FPK_WB = 256          # wbT [128, 16]
FPK_WC = 272          # wcT [128, 16]
FPK_COLS = 288        # 12 bias/scale columns
COL_SC1P, COL_BI1P, COL_SC1C, COL_BI1C = 0, 1, 2, 3
COL_SC2P, COL_BI2P, COL_SC2C, COL_BI2C = 4, 5, 6, 7
COL_BB, COL_BC, COL_ALPHA, COL_ABD, COL_BETA = 8, 9, 10, 11, 12
FPK_F = 288 + 13


def build_nc(variant="full"):
    nc = bass.Bass("TRN2", num_devices=NCORES)

    # -------- parameters (per-core views, host-packed) --------
    xs = nc.declare_dram_parameter("xs", [CIN, XR, WP], BF16, isOutput=False)
    w1p = nc.declare_dram_parameter("w1p", [CIN, 9 * C1], FP32, isOutput=False)
    w1c = nc.declare_dram_parameter("w1c", [CIN, 9 * C1], FP32, isOutput=False)
    w2pk = nc.declare_dram_parameter("w2pk", [C1, 2 * 9 * CO], FP32, isOutput=False)
    fpk = nc.declare_dram_parameter("fpk", [C1, FPK_F], FP32, isOutput=False)
    emask = nc.declare_dram_parameter("emask", [NQ], BF16, isOutput=False)
    outp = nc.declare_dram_parameter("out", [CO, 32, W], BF16, isOutput=True)

    with tile.TileContext(nc) as tc:
        px = tc.alloc_tile_pool(name="px", bufs=4)
        pw = tc.alloc_tile_pool(name="pw", bufs=1)
        pio = tc.alloc_tile_pool(name="pio", bufs=1)
        pe3 = tc.alloc_tile_pool(name="pe3", bufs=4)
        psm = tc.alloc_tile_pool(name="psm", bufs=2)
        dram = tc.alloc_tile_pool(name="dram", bufs=2, space="DRAM")
        # PSUM plan (8 banks): sp 2x[128,1024] (4) for conv chunks / S pairs /
        # conv2; u 2 (up accum); q 1 (q/k/z); m 1 (transposes/gram/dp/zbs/cam)
        ps_sp = tc.alloc_tile_pool(name="ps_sp", bufs=2, space="PSUM")
        ps_u = tc.alloc_tile_pool(name="ps_u", bufs=2, space="PSUM")
        ps_q = tc.alloc_tile_pool(name="ps_q", bufs=1, space="PSUM")
        ps_m = tc.alloc_tile_pool(name="ps_m", bufs=1, space="PSUM")

        # ---------------- loads ------------------------------------------
        # x and w1p ship as bf16 (half the front-loaded DMA bytes) and are
        # widened to fp32r on the otherwise-idle Pool engine, pipelined with
        # the DMA stream so conv_p1 starts ~4us in.
        x_sb = [
            px.tile([C1, XR, WP], FP32R, tag="xt", name=f"x{i}")
            for i in range(4)
        ]
        w1p_sb, w1c_sb = [], []

        def xload(c4, r0):
            st = pe3.tile([C1, 12, WP], BF16, tag="xstg", name=f"xs{c4}_{r0}")
            nc.sync.dma_start(
                out=st[:], in_=xs[c4 * C1:(c4 + 1) * C1, r0:r0 + 12, :]
            )
            nc.gpsimd.tensor_copy(
                out=x_sb[c4][:, r0:r0 + 12, :], in_=st[:]
            )

        for c4 in range(4):
            xload(c4, 0)
            t = pw.tile([C1, 9, C1], FP32R, tag=f"w1p{c4}")
            nc.sync.dma_start(
                out=t[:], in_=w1p[c4 * C1:(c4 + 1) * C1, :].bitcast(FP32R)
            )
            w1p_sb.append(t)
        for r0 in (12, 24):
            for c4 in range(4):
                xload(c4, r0)

        # w1c / w2 / fpk on vector queue (DVE idle early)
        for c4 in range(4):
            t = pw.tile([C1, 9, C1], FP32R, tag=f"w1c{c4}")
            nc.scalar.dma_start(
                out=t[:], in_=w1c[c4 * C1:(c4 + 1) * C1, :].bitcast(FP32R)
            )
            w1c_sb.append(t)
        w2_sb = pw.tile([C1, 2, 9, CO], FP32R, tag="w2")
        nc.scalar.dma_start(
            out=w2_sb[:],
            in_=w2pk[:].bitcast(FP32R).rearrange(
                "p (a t o) -> p a t o", a=2, t=9
            ),
        )
        fpk_sb = pw.tile([C1, FPK_F], FP32R, tag="fpk")
        nc.scalar.dma_start(out=fpk_sb[:], in_=fpk[:].bitcast(FP32R))

        # mask broadcast on gpsimd
        maskb = pw.tile([C1, NQ], BF16, tag="mask")
        nc.gpsimd.dma_start(
            out=maskb[:],
            in_=bass.AP(tensor=emask[:].tensor, offset=0, ap=[[0, C1], [1, NQ]]),
        )

        wd_sb = fpk_sb[:, FPK_WD:FPK_WD + C1]
        id_sb = fpk_sb[:, FPK_ID:FPK_ID + C1].bitcast(FP32)
        wb_sb = fpk_sb[:, FPK_WB:FPK_WB + C8]
        wc_sb = fpk_sb[:, FPK_WC:FPK_WC + C8]

        def col(i, n=C1):
            return fpk_sb[0:n, FPK_COLS + i:FPK_COLS + i + 1].bitcast(FP32)

        # on-chip constants
        ones_col = pw.tile([C1, 1], BF16, tag="onec")
        nc.vector.memset(ones_col[:], 1.0)
        ones_row = pw.tile([1, C1], BF16, tag="oner")
        nc.vector.memset(ones_row[:], 1.0)

        # main activation buffers (un-padded ext layout)
        ypx = pio.tile([C1, PR, W], FP32R, tag="ypx")
        ycx = pio.tile([C1, PR, W], FP32R, tag="ycx")
        ypf = ypx.rearrange("p r c -> p (r c)")
        ycf = ycx.rearrange("p r c -> p (r c)")

        # padded conv2 inputs
        p_pad = pio.tile([C1, PR, WP], FP32R, tag="ppad")
        c_pad = pio.tile([C1, PR, WP], FP32R, tag="cpad")
        for t_ in (p_pad, c_pad):
            nc.vector.memset(t_[:, :, 0:1].bitcast(FP32), 0.0)
            nc.vector.memset(t_[:, :, WP - 1:WP].bitcast(FP32), 0.0)
        p_flat = p_pad.rearrange("p r c -> p (r c)")
        c_flat = c_pad.rearrange("p r c -> p (r c)")

        k_sb = pio.tile([C8, NFULL], FP32R, tag="ksb")
        dT8_sb = pio.tile([C1, 32, C1], F8, tag="dT8")
        dT_sb = pio.tile([C1, 32, C1], BF16, tag="dT")

        # DRAM bounce buffers for collectives
        mh_in0 = dram.tile([MH_BYTES], F8, tag="mhi0")
        mh_in1 = dram.tile([MH_BYTES], F8, tag="mhi1")
        mh_out0 = dram.tile([2 * MH_BYTES], F8, tag="mho0")
        mh_out1 = dram.tile([2 * MH_BYTES], F8, tag="mho1")
        mh_in = [mh_in0, mh_in1]
        mh_out = [mh_out0, mh_out1]
        gb_in = dram.tile([C1, C1], FP32, tag="gbi")
        gb_out = dram.tile([2 * C1, C1], FP32, tag="gbo")

        def cpv_flat(cp, slot, n2, np_=C1):
            b = cp.rearrange("p a b -> p (a b)")
            v = b[:, slot * 512:slot * 512 + n2]
            if np_ != C1:
                v = bass.AP(tensor=v.tensor, offset=v.offset,
                            ap=[[v.ap[0][0], np_]] + list(v.ap[1:]))
            return v

        def cpv_rows(cp, slot, nr, np_=C1):
            b = cp[:]
            return bass.AP(
                tensor=b.tensor, offset=b.offset + slot * 512,
                ap=[[b.ap[0][0], np_], [WP, nr], [1, W]],
            )

        xf = [t.rearrange("p r c -> p (r c)") for t in x_sb]

        def conv1(w_sb, sc, bi, dst, chunks):
            for (r0, nr) in chunks:
                n2 = nr * WP - 2
                cp = ps_sp.tile([C1, 2, 512], FP32, tag="sp", name=f"cv{r0}")
                first = True
                for c4 in range(4):
                    for t in range(9):
                        ky, kx = t // 3, t % 3
                        off = (r0 + ky) * WP + kx
                        nc.tensor.matmul(
                            cpv_flat(cp, 0, n2),
                            w_sb[c4][:, t, :],
                            xf[c4][:, off:off + n2],
                            start=first,
                            stop=(c4 == 3 and t == 8),
                        )
                        first = False
                nc.scalar.activation(
                    dst[:, r0:r0 + nr, :], cpv_rows(cp, 0, nr),
                    AF.Relu, bias=bi, scale=sc,
                )

        # ---------------- conv_p1 + kd (half-interleaved) ----------------
        k_loc = pio.tile([C8, NLOC], FP32R, tag="kloc")
        dT_loc = pio.tile([C1, 16, C1], F8, tag="dTloc")

        def kd_half(hh):
            # local rows 1+16*hh .. 16+16*hh = ypx flat 64+1024*hh ..
            for j in (0, 1):
                i = 2 * hh + j
                kp = ps_q.tile([C8, 512], FP32, tag="q")
                nc.tensor.matmul(
                    kp[:], wc_sb, ypf[:, W + 512 * i:W + 512 * (i + 1)],
                    start=True, stop=True,
                )
                nc.scalar.activation(
                    k_loc[:, 512 * i:512 * (i + 1)], kp[:], AF.Identity,
                    bias=col(COL_BC, C8),
                )
            # dT blocks [pos 128, ch 128] in fp8 (no bias: alpha*bd folded
            # into the PAM epilogue)
            for t in range(8 * hh, 8 * hh + 8):
                dp = ps_m.tile([C1, C1], FP32, tag="m")
                nc.tensor.matmul(
                    dp[:], ypf[:, W + C1 * t:W + C1 * (t + 1)], wd_sb,
                    start=True, stop=True,
                )
                nc.scalar.activation(dT_loc[:, t, :], dp[:], AF.Identity)
            nc.scalar.dma_start(
                out=mh_in[hh][0:DT_BYTES].rearrange("(p x) -> p x", p=C1),
                in_=dT_loc[:, 8 * hh:8 * hh + 8, :].rearrange(
                    "p a b -> p (a b)"
                ),
            )
            nc.scalar.dma_start(
                out=mh_in[hh][DT_BYTES:MH_BYTES].bitcast(FP32R).rearrange(
                    "(p x) -> p x", p=C8
                ),
                in_=k_loc[:, 1024 * hh:1024 * (hh + 1)],
            )
            if variant != "noccl":
                nc.gpsimd.collective_compute(
                    "AllGather",
                    mybir.AluOpType.bypass,
                    replica_groups=PAIRS,
                    ins=[mh_in[hh][:].opt()],
                    outs=[mh_out[hh][:].opt()],
                )
            else:
                nc.sync.dma_start(out=mh_out[hh][:MH_BYTES], in_=mh_in[hh][:])
                nc.sync.dma_start(out=mh_out[hh][MH_BYTES:], in_=mh_in[hh][:])
            # gathered loads: core h's half hh
            mho = mh_out[hh][:]
            mhof = mh_out[hh][:].bitcast(FP32R)
            for h in range(2):
                base = h * MH_BYTES
                nc.sync.dma_start(
                    out=dT8_sb[:, 16 * h + 8 * hh:16 * h + 8 * hh + 8, :],
                    in_=bass.AP(
                        tensor=mho.tensor, offset=mho.offset + base,
                        ap=[[8 * C1, C1], [1, 8 * C1]],
                    ),
                )
                nc.sync.dma_start(
                    out=k_sb[:, NLOC * h + 1024 * hh:
                             NLOC * h + 1024 * (hh + 1)],
                    in_=bass.AP(
                        tensor=mhof.tensor,
                        offset=mhof.offset + (base + DT_BYTES) // 4,
                        ap=[[1024, C8], [1, 1024]],
                    ),
                )
                nc.gpsimd.tensor_copy(
                    out=dT_sb[:, 16 * h + 8 * hh:16 * h + 8 * hh + 8, :]
                    .rearrange("p a b -> p (a b)"),
                    in_=dT8_sb[:, 16 * h + 8 * hh:16 * h + 8 * hh + 8, :]
                    .rearrange("p a b -> p (a b)"),
                )

        conv1(w1p_sb, col(COL_SC1P), col(COL_BI1P), ypx, CHR[:3])
        kd_half(0)
        conv1(w1p_sb, col(COL_SC1P), col(COL_BI1P), ypx, CHR[3:])
        kd_half(1)

        # ---------------- conv_c1 + gram ----------------
        conv1(w1c_sb, col(COL_SC1C), col(COL_BI1C), ycx, CHR)

        aT_sb = pio.tile([C1, 16, C1], FP32R, tag="aTsb")
        for t in range(16):
            tp = ps_m.tile([C1, C1], FP32, tag="m")
            nc.tensor.transpose(
                tp[:], ycf[:, W + C1 * t:W + C1 * (t + 1)].bitcast(FP32), id_sb
            )
            nc.vector.tensor_copy(out=aT_sb[:, t, :], in_=tp[:])
        gp = ps_m.tile([C1, C1], FP32, tag="m")
        for t in range(16):
            nc.tensor.matmul(
                gp[:], aT_sb[:, t, :], aT_sb[:, t, :],
                start=(t == 0), stop=(t == 15),
            )
        g_loc = pio.tile([C1, C1], FP32, tag="gloc")
        nc.vector.tensor_copy(out=g_loc[:], in_=gp[:])
        nc.sync.dma_start(out=gb_in[:], in_=g_loc[:])
        if variant != "noccl":
            nc.gpsimd.collective_compute(
                "AllGather",
                mybir.AluOpType.bypass,
                replica_groups=PAIRS,
                ins=[gb_in[:].opt()],
                outs=[gb_out[:].opt()],
            )
        else:
            nc.sync.dma_start(out=gb_out[:C1, :], in_=gb_in[:])
            nc.sync.dma_start(out=gb_out[C1:, :], in_=gb_in[:])



        # ---------------- PAM ----------------
        hp = tc.high_priority()
        hp.__enter__()
        for (s, n) in CHP:
            r0, rr = s // W, n // W
            qp = ps_q.tile([C8, 512], FP32, tag="q")
            nc.tensor.matmul(
                qp[:, :n], wb_sb, ypf[:, s:s + n], start=True, stop=True
            )
            q_sb = psm.tile([C8, 512], FP32R, tag="qsb")
            nc.vector.tensor_scalar_add(
                out=q_sb[:, :n], in0=qp[:, :n], scalar1=col(COL_BB, C8)
            )

            zacc = psm.tile([C1, 2, 512], BF16, tag="zacc")
            up = ps_u.tile([C1, 512], FP32, tag="u")
            mp_first = True
            for mp in (0, 1, 2, 3, 8, 9, 10, 11, 4, 5, 6, 7, 12, 13, 14, 15):
                sp = ps_sp.tile([C1, 2, 512], FP32, tag="sp")
                for h in range(2):
                    nc.tensor.matmul(
                        sp[:, h, :n],
                        k_sb[:, (2 * mp + h) * C1:(2 * mp + h + 1) * C1],
                        q_sb[:, :n],
                        start=True,
                        stop=True,
                    )
                et = pe3.tile([C1, 2, 512], BF16, tag="et")
                nc.scalar.activation(et[:, :, :n], sp[:, :, :n], AF.Exp)
                if mp_first:
                    nc.vector.tensor_copy(out=zacc[:, :, :n], in_=et[:, :, :n])
                else:
                    nc.vector.tensor_add(
                        out=zacc[:, :, :n], in0=zacc[:, :, :n], in1=et[:, :, :n]
                    )
                for h in range(2):
                    nc.tensor.matmul(
                        up[:, :n],
                        dT_sb[:, 2 * mp + h, :],
                        et[:, h, :n],
                        start=(mp_first and h == 0),
                        stop=(mp == 15 and h == 1),
                    )
                mp_first = False
            nc.vector.tensor_add(
                out=zacc[:, 0, :n], in0=zacc[:, 0, :n], in1=zacc[:, 1, :n]
            )
            zp = ps_q.tile([1, 512], FP32, tag="q")
            nc.tensor.matmul(
                zp[:, :n], ones_col[:], zacc[:, 0, :n], start=True, stop=True
            )
            zr = psm.tile([1, 512], FP32, tag="zr")
            nc.vector.reciprocal(out=zr[:, :n], in_=zp[:, :n])
            zrb = psm.tile([1, 512], BF16, tag="zrb")
            nc.vector.tensor_scalar_mul(
                out=zrb[:, :n], in0=zr[:, :n], scalar1=col(COL_ALPHA, 1)
            )
            zbp = ps_m.tile([C1, 512], FP32, tag="m")
            nc.tensor.matmul(
                zbp[:, :n], ones_row[:], zrb[:, :n], start=True, stop=True
            )
            zbs = psm.tile([C1, 512], BF16, tag="zbs")
            nc.vector.tensor_copy(out=zbs[:, :n], in_=zbp[:, :n])
            t1 = psm.tile([C1, 512], FP32, tag="t1")
            nc.vector.tensor_mul(out=t1[:, :n], in0=up[:, :n], in1=zbs[:, :n])
            nc.vector.tensor_scalar(
                out=t1[:, :n], in0=t1[:, :n],
                scalar1=col(COL_ABD), scalar2=None,
                op0=mybir.AluOpType.add,
            )
            nc.vector.tensor_add(
                out=t1[:, :n], in0=t1[:, :n], in1=ypf[:, s:s + n].bitcast(FP32)
            )
            nc.vector.tensor_mul(
                out=p_pad[:, r0:r0 + rr, 1:1 + W],
                in0=t1[:, :n].rearrange("p (r c) -> p r c", c=W),
                in1=maskb[:, s:s + n].rearrange("p (r c) -> p r c", c=W),
            )
        hp.__exit__(None, None, None)

        # ---- conv_p2 (overlaps PAM tail) ----
        out_sb = pio.tile([CO, PR, WP], BF16, tag="outsb")
        for (r0, nr) in CH2:
            n2 = nr * WP - 2
            o1p = ps_sp.tile([C1, 2, 512], FP32, tag="sp", name=f"p2{r0}")
            for t in range(9):
                ky, kx = t // 3, t % 3
                off = (r0 + ky) * WP + kx
                nc.tensor.matmul(
                    cpv_flat(o1p, 0, n2, CO),
                    w2_sb[:, 0, t, :],
                    p_flat[:, off:off + n2],
                    start=(t == 0),
                    stop=(t == 8),
                )
            nc.scalar.activation(
                out_sb[:, r0:r0 + nr, 1:1 + W], cpv_rows(o1p, 0, nr, CO),
                AF.Relu, bias=col(COL_BI2P, CO), scale=col(COL_SC2P, CO),
            )

        # ---------------- CAM ----------------
        g_full = pio.tile([C1, C1], FP32, tag="gfull")
        g_peer = pio.tile([C1, C1], FP32, tag="gpeer")
        nc.sync.dma_start(out=g_full[:], in_=gb_out[:C1, :])
        nc.sync.dma_start(out=g_peer[:], in_=gb_out[C1:, :])
        nc.vector.tensor_add(out=g_full[:], in0=g_full[:], in1=g_peer[:])
        rowmax = pio.tile([C1, 1], FP32, tag="rmax")
        nc.vector.tensor_reduce(
            out=rowmax[:], in_=g_full[:], op=mybir.AluOpType.min,
            axis=mybir.AxisListType.X,
        )
        gdiff = pio.tile([C1, C1], FP32, tag="gdiff")
        nc.vector.tensor_scalar(
            out=gdiff[:], in0=g_full[:], scalar1=rowmax[:], scalar2=None,
            op0=mybir.AluOpType.subtract,
        )
        nc.vector.tensor_scalar_min(out=gdiff[:], in0=gdiff[:], scalar1=80.0)
        gexp = pio.tile([C1, C1], FP32, tag="gexp")
        nc.scalar.activation(gexp[:], gdiff[:], AF.Exp, scale=-1.0)
        rowsum = pio.tile([C1, 1], FP32, tag="rsum")
        nc.vector.reduce_sum(out=rowsum[:], in_=gexp[:], axis=mybir.AxisListType.X)
        rinv = pio.tile([C1, 1], FP32, tag="rinv")
        nc.vector.reciprocal(out=rinv[:], in_=rowsum[:])
        attn = pio.tile([C1, C1], FP32, tag="attn")
        nc.vector.tensor_scalar_mul(out=attn[:], in0=gexp[:], scalar1=rinv[:])
        nc.vector.tensor_scalar_mul(
            out=attn[:], in0=attn[:], scalar1=col(COL_BETA)
        )
        atp = ps_m.tile([C1, C1], FP32, tag="m")
        nc.tensor.transpose(atp[:], attn[:], id_sb)
        attnT = pio.tile([C1, C1], FP32R, tag="attnTs")
        nc.vector.tensor_copy(out=attnT[:], in_=atp[:])

        for (s, n) in CHP:
            r0, rr = s // W, n // W
            cm = ps_m.tile([C1, 512], FP32, tag="m")
            nc.tensor.matmul(
                cm[:, :n], attnT[:], ycf[:, s:s + n], start=True, stop=True
            )
            t3 = psm.tile([C1, 512], FP32, tag="t1")
            nc.vector.tensor_add(
                out=t3[:, :n], in0=cm[:, :n], in1=ycf[:, s:s + n].bitcast(FP32)
            )
            nc.vector.tensor_mul(
                out=c_pad[:, r0:r0 + rr, 1:1 + W],
                in0=t3[:, :n].rearrange("p (r c) -> p r c", c=W),
                in1=maskb[:, s:s + n].rearrange("p (r c) -> p r c", c=W),
            )

        # ---------------- conv_c2 + final add + store ----------------
        for (r0, nr) in CH2:
            n2 = nr * WP - 2
            o2p = ps_sp.tile([C1, 2, 512], FP32, tag="sp", name=f"c2{r0}")
            for t in range(9):
                ky, kx = t // 3, t % 3
                off = (r0 + ky) * WP + kx
                nc.tensor.matmul(
                    cpv_flat(o2p, 0, n2, CO),
                    w2_sb[:, 1, t, :],
                    c_flat[:, off:off + n2],
                    start=(t == 0),
                    stop=(t == 8),
                )
            o2 = psm.tile([CO, 7, W], FP32, tag="o2s")
            nc.scalar.activation(
                o2[:, :nr, :], cpv_rows(o2p, 0, nr, CO),
                AF.Relu, bias=col(COL_BI2C, CO), scale=col(COL_SC2C, CO),
            )
            nc.vector.tensor_add(
                out=out_sb[:, r0:r0 + nr, 1:1 + W],
                in0=out_sb[:, r0:r0 + nr, 1:1 + W], in1=o2[:, :nr, :],
            )
            nc.sync.dma_start(
                out=outp[:, r0:r0 + nr, :],
                in_=out_sb[:, r0:r0 + nr, 1:1 + W],
            )

        for p in (ps_m, ps_q, ps_u, ps_sp, dram, psm, pe3, pio, pw, px):
            p.release()

    legalize_single_wait(nc)

    # The neuron compile cache keys on the HLO, which does NOT include the
    # bass_exec backend_config (the BIR). Declare an unused input whose SHAPE
    # encodes a hash of the built module so any kernel change produces a new
    # cache key instead of silently reusing a stale NEFF.
    import hashlib
    h = int.from_bytes(
        hashlib.sha256(nc.to_json_bytes()).digest()[:4], "little"
    )
    nonce_len = 1 + (h % 4096)
    nc.declare_dram_parameter("nonce", [nonce_len], FP32, isOutput=False)
    nc._nonce_len = nonce_len
    return nc


def pack_inputs(inputs):
    """Host-side packing: per-core input maps."""
    import ml_dtypes
    bf16 = ml_dtypes.bfloat16
    x = np.asarray(inputs["x"], dtype=np.float32)

    def t1(w, dt=np.float32):
        # [O,CI,3,3] -> [CI, 9*O] with layout [ci][ky*3+kx][o]
        w = np.asarray(w, dtype=np.float32)
        o = w.shape[0]
        return np.ascontiguousarray(
            w.transpose(1, 2, 3, 0).reshape(w.shape[1], 9 * o)
        ).astype(dt)

    def bnsb(g, b, m, v):
        g, b, m, v = (np.asarray(a, dtype=np.float32) for a in (g, b, m, v))
        sc = g / np.sqrt(v + EPS)
        return sc, b - m * sc

    sc1p_, bi1p_ = bnsb(inputs["gp1"], inputs["bp1"], inputs["mp1"], inputs["vp1"])
    sc1c_, bi1c_ = bnsb(inputs["gc1"], inputs["bc1"], inputs["mc1"], inputs["vc1"])
    sc2p_, bi2p_ = bnsb(inputs["gp2"], inputs["bp2"], inputs["mp2"], inputs["vp2"])
    sc2c_, bi2c_ = bnsb(inputs["gc2"], inputs["bc2"], inputs["mc2"], inputs["vc2"])

    w2p = (np.asarray(inputs["wp2"], np.float32)
           .transpose(1, 2, 3, 0).reshape(C1, 9 * CO))
    w2c = (np.asarray(inputs["wc2"], np.float32)
           .transpose(1, 2, 3, 0).reshape(C1, 9 * CO))
    w2pk = np.concatenate([w2p, w2c], axis=1)

    alpha = float(np.asarray(inputs["alpha"], np.float32)[0])
    beta = float(np.asarray(inputs["beta"], np.float32)[0])
    bd = np.asarray(inputs["pam_bd"], np.float32)

    fpk = np.zeros((C1, FPK_F), np.float32)
    fpk[:, FPK_WD:FPK_WD + C1] = np.asarray(inputs["pam_wd"], np.float32).T
    fpk[:, FPK_ID:FPK_ID + C1] = np.eye(C1, dtype=np.float32)
    fpk[:, FPK_WB:FPK_WB + C8] = np.asarray(inputs["pam_wb"], np.float32).T
    fpk[:, FPK_WC:FPK_WC + C8] = np.asarray(inputs["pam_wc"], np.float32).T

    def setcol(i, v, n=C1):
        fpk[:n, FPK_COLS + i] = v

    setcol(COL_SC1P, sc1p_)
    setcol(COL_BI1P, bi1p_)
    setcol(COL_SC1C, sc1c_)
    setcol(COL_BI1C, bi1c_)
    setcol(COL_SC2P, sc2p_, CO)
    setcol(COL_BI2P, bi2p_, CO)
    setcol(COL_SC2C, sc2c_, CO)
    setcol(COL_BI2C, bi2c_, CO)
    setcol(COL_BB, np.asarray(inputs["pam_bb"], np.float32), C8)
    setcol(COL_BC, np.asarray(inputs["pam_bc"], np.float32), C8)
    setcol(COL_ALPHA, alpha)
    setcol(COL_ABD, alpha * bd)
    setcol(COL_BETA, beta)

    shared = {
        "w1p": t1(inputs["wp1"]),
        "w1c": t1(inputs["wc1"]),
        "w2pk": w2pk,
        "fpk": fpk,
    }

    in_maps = []
    for core in range(NCORES):
        b, hf = core // 2, core % 2
        xsl = np.zeros((CIN, XR, WP), np.float32)
        if hf == 0:
            xsl[:, 2:36, 1:65] = x[b, :, 0:34, :]
        else:
            xsl[:, 0:34, 1:65] = x[b, :, 30:64, :]
        em = np.zeros((PR, W), np.float32)
        if hf == 0:
            em[1:34, :] = 1.0
        else:
            em[0:33, :] = 1.0
        m = dict(shared)
        m["xs"] = xsl.astype(bf16)
        m["emask"] = em.reshape(-1).astype(bf16)
        in_maps.append(m)
    return in_maps


def unpack_outputs(results):
    out = np.zeros((B, CO, H, W), np.float32)
    for core in range(NCORES):
        b, hf = core // 2, core % 2
        out[b, :, hf * 32:(hf + 1) * 32, :] = np.asarray(
            results[core]["out"], dtype=np.float32
        )
    return out


_NC_CACHE = [None]


def kernel(**inputs) -> np.ndarray:
    # the axon NTFF trace hook module is absent here; make sure a stray
    # BASS_TRACE env var cannot route run_bass_kernel_spmd into it
    os.environ["BASS_NEVER_TRACE"] = "1"
    if _NC_CACHE[0] is None:
        _NC_CACHE[0] = build_nc()
    nc = _NC_CACHE[0]
    in_maps = pack_inputs(inputs)
    nz = np.zeros([getattr(nc, "_nonce_len", 1)], np.float32)
    for m in in_maps:
        m["nonce"] = nz
    # First executions through the axon PJRT path very occasionally return
    # non-finite garbage (infra-level flake: the steady-state reruns of the
    # same NEFF produce correct values). Detect and re-dispatch.
    out = None
    for _ in range(3):
        res = run_bass_kernel_spmd(nc, in_maps, list(range(NCORES)), trace=False)
        out = unpack_outputs(res.results)
        if np.isfinite(out).all():
            break
    return out
